# revision 34
# baseline (speedup 1.0000x reference)
"""CRF forward (logsumexp over paths) loss kernel for Trainium2, 8 NeuronCores.

Math
----
reference:  fv0 = alpha_0^T + emits[0]                       [B, K]
            fv_t[b,j] = logsumexp_i(fv_{t-1}[b,i] + trans[i,j]) + emit_t[b,j]
            alpha_z = sum_b logsumexp_k( fv_{tau_b}[b,:] )   (tau = one-hot mask step)

We run the recurrence in exp space.  With ETs[i,j] = exp(trans[i,j] - delta)
and e_t[j,b] = exp(emit_t[b,j]) (transposed), the state w_t[j,b] =
exp(fv_t[j,b] - delta*t - C[b]) obeys

    w_t = (ETs^T w_{t-1}) * e_t        (one matmul + one elementwise mul)

The serial chain is latency-bound (PE->DVE->PE round trip per step), so the
whole design minimizes per-step work off that path:

 * emissions are transposed ON THE HOST to [K, T, B] so the per-step
   emission tile arrives from HBM already in state layout: no PE transposes,
   no staging copies.  The exp() runs on ACT one window ahead, straight into
   a 3-window SBUF ring whose 65th row is preset to 1.0 once.
 * the transition matrix is augmented with a 65th column of ones so each
   matmul also emits colsum(w_{t-1}) into PSUM row 64; the fused DVE
   multiply covers 65 rows, capturing every step's column sum into the state
   ring for free.  One gpsimd DMA per window archives the row into a colsum
   history buffer.
 * renorm (every RP*W=128 steps): ln/exp on ACT from a stale colsum, a tiny
   bf16 broadcast matmul builds the scale tile, and the scale is
   pre-multiplied into the GROUP-END emission tile (rows 0:64 only) off the
   critical path, so the boundary step is an ordinary step.  log(scale) is
   booked into a C history row buffer (gpsimd tensor ops, one late DMA).
 * the final combine is precomputed on the idle gpsimd engine during the
   chain (mask reductions, C terms, and the masked colsum sum for windows
   0..30); only the last window's contribution runs after the chain, feeding
   a PSUM-accumulating matmul finisher.

The one-hot time mask turns "select alpha at tau_b" into a linear masked sum
over the colsum history:

    result[b] = log( sum_s mask[s-1,b] * colsum_{s-1}[b] ) + C_win(s) + delta*tau_b

Sharding: batch B=512 split across 8 cores (64 per core); transitions/alpha_0
replicated; final alpha_z = host sum of the 8 per-core [1,64] row outputs.
"""

import os
import sys

for _p in ("/opt/trn_rl_repo", "/root/.axon_site/_ro/trn_rl_repo"):
    if os.path.isdir(_p) and _p not in sys.path:
        sys.path.insert(0, _p)

from contextlib import ExitStack

import numpy as np

import concourse.bass as bass
import concourse.mybir as mybir
import concourse.tile as tile
from concourse.bass_utils import run_bass_kernel_spmd

# The walrus build in this container rejects instructions carrying more than
# one sync-wait command ("Too many sync wait commands" in setupSyncWait).
# Tile freely emits multi-wait instructions, so split the extras onto
# preceding same-engine no-ops at commit time (engine queues execute
# in-order, so the semantics are identical).
_ORIG_COMMIT = tile.TileContext._commit_instruction
_PROD_SEMS: dict = {}
_PARTNER = {}


def _single_wait_commit(self, inst, lazy_reg_writes=True):
    si = getattr(inst, "sync_info", None)
    eng_t = inst.engine
    if si is not None and eng_t != mybir.EngineType.Unassigned and si.on_update:
        s = _PROD_SEMS.setdefault(eng_t, set())
        for u in si.on_update:
            s.add(u.id)
    if (
        si is not None
        and si.on_wait
        and len(si.on_wait) > 1
        and eng_t != mybir.EngineType.Unassigned
    ):
        # keep the wait produced by the partner engine of the serial chain
        # (PE<->DVE) on the instruction itself: a NOP-carried data wait adds
        # ~100ns of wake+dispatch to every chain hop
        waits = list(si.on_wait)
        keep = 0
        partner = _PARTNER.get(eng_t)
        if partner is not None:
            psems = _PROD_SEMS.get(partner, ())
            for j, w in enumerate(waits):
                if w.id in psems:
                    keep = j
                    break
        kw = waits.pop(keep)
        eng = self.nc.engines[eng_t]
        for w in waits:
            n = eng.nop(nofuse=True)
            n.ins.sync_info = mybir.SyncInfo(on_wait=[w], on_update=[])
        inst.sync_info = mybir.SyncInfo(
            on_wait=[kw], on_update=list(si.on_update or [])
        )
    _ORIG_COMMIT(self, inst, lazy_reg_writes)


_PARTNER[mybir.EngineType.PE] = mybir.EngineType.DVE
_PARTNER[mybir.EngineType.DVE] = mybir.EngineType.PE


tile.TileContext._commit_instruction = _single_wait_commit

T, B, K = 512, 512, 64
NCORES = 8
BSH = B // NCORES          # 64 batch elements per core
W = 16                     # slots per window (colsum capture period)
RP = 8                     # renorm every RP windows (64 steps: the per-b
                           # log-magnitude walk stays within ~ +-50
                           # nats, inside bf16/fp32 range)
NWINCHAIN = T // W         # 32 windows of chain steps (slots 0..511)
NWIN = NWINCHAIN + 1       # 33: slot 512 (colsum of t=511) lands in window 32
# window -> row mapping in the [NWINR, .] combine tensors: compute-engine ops
# need 32-aligned partition bases, so the two LATE windows sit in the aligned
# 32..33 block (win 31 -> row 33 via DMA capture, win 32 -> row 32 via ACT
# copy); row 31 stays zero and contributes nothing.
NWINR = NWIN + 1           # 34 rows
ROW_W31 = 33
ROW_W32 = 32
HB = BSH // 2              # sub-chain half width
NREM = 4                   # raw-emission ring windows
NRET = 4                   # exp-emission ring windows (2-window exp
                           # lead + 1 window of WAR slack at reuse)
DELTA = 5.0                # static per-step log-space offset folded into ETs
F32 = mybir.dt.float32
BF16 = mybir.dt.bfloat16
U8 = mybir.dt.uint8
I32 = mybir.dt.int32
MULT = mybir.AluOpType.mult
ADD = mybir.AluOpType.add
AX = mybir.AxisListType.X
AF = mybir.ActivationFunctionType


def _build_crf_nc() -> bass.Bass:
    nc = bass.Bass(trn_type="TRN2", target_bir_lowering=False, debug=False)

    emits_d = nc.dram_tensor("emits_t", [K, T, BSH], F32, kind="ExternalInput").ap()
    mask_d = nc.dram_tensor("maskb", [T, BSH], U8, kind="ExternalInput").ap()
    trans_d = nc.dram_tensor("transitions", [K, K], F32, kind="ExternalInput").ap()
    alpha0_d = nc.dram_tensor("alpha_0", [K, 1], F32, kind="ExternalInput").ap()
    out_d = nc.dram_tensor("out_row", [1, BSH], F32, kind="ExternalOutput").ap()

    with tile.TileContext(nc) as tc:
        with ExitStack() as ctx:
            _crf_body(ctx, tc, emits_d, mask_d, trans_d, alpha0_d, out_d)
    _split_remaining_multiwaits(nc)
    return nc


def _split_remaining_multiwaits(nc):
    """Split multi-wait instructions added outside the commit path (e.g. the
    end-of-kernel drain/barrier) onto preceding same-engine no-ops."""
    for blk in nc.m.functions[0].blocks:
        il = blk.instructions
        idx = 0
        while idx < len(il):
            inst = il[idx]
            si = inst.sync_info
            if si is not None and si.on_wait and len(si.on_wait) > 1:
                waits = list(si.on_wait)
                for j, w in enumerate(waits[:-1]):
                    n = mybir.InstNoOp(
                        name=f"I-swx-{inst.name}-{j}", ins=[], outs=[]
                    )
                    n.engine = inst.engine
                    n.sync_info = mybir.SyncInfo(on_wait=[w], on_update=[])
                    nc.register_instruction(n, overwrite=True)
                    il.insert(idx, n)
                    idx += 1
                inst.sync_info = mybir.SyncInfo(
                    on_wait=[waits[-1]], on_update=list(si.on_update or [])
                )
            idx += 1


def _crf_body(ctx, tc, emits_d, mask_d, trans_d, alpha0_d, out_d):
    nc = tc.nc

    # ---- long-lived SBUF state ----
    ets = nc.alloc_sbuf_tensor("ets", [K, K + 1], BF16).ap()        # exp(trans-d)|1
    # chain state ring: 2 window buffers x W slots x BSH cols, 65 rows
    # (row 64 of slot s = colsum of w_{s-1})
    w_all = nc.alloc_sbuf_tensor("w_all", [K + 1, 2 * W * BSH], BF16).ap()
    # exp-emission ring, already transposed on host: slot t -> e_t[k, b];
    # row 64 is constant 1.0 (preset once) so the fused multiply also
    # captures the colsum row.
    etr = nc.alloc_sbuf_tensor("etr", [K + 1, NRET * W * BSH], BF16).ap()
    em = nc.alloc_sbuf_tensor("em", [K, NREM * W * BSH], F32).ap()  # raw slabs
    csum = nc.alloc_sbuf_tensor("csum", [NWINR, W * BSH], BF16).ap()  # colsum hist
    c_hist = nc.alloc_sbuf_tensor("c_hist", [NWINR, BSH], F32).ap()  # log-norm/win
    c_histT = nc.alloc_sbuf_tensor("c_histT", [1, NWINR * BSH], F32).ap()
    zrow = nc.alloc_sbuf_tensor("zrow", [1, BSH], F32).ap()
    c_rows = nc.alloc_sbuf_tensor("c_rows", [1, 2 * BSH], F32).ap()  # C ping-pong
    maskw = nc.alloc_sbuf_tensor("maskw", [NWINR, W * BSH], F32).ap()
    mk_u8 = nc.alloc_sbuf_tensor("mk_u8", [NWINR, W * BSH], U8).ap()
    iota_i = nc.alloc_sbuf_tensor("iota_i", [NWINR, W * BSH], I32).ap()
    iotaw = nc.alloc_sbuf_tensor("iotaw", [NWINR, W * BSH], F32).ap()
    ones_r = nc.alloc_sbuf_tensor("ones_r", [1, K], BF16).ap()     # bc lhsT
    ones_c = nc.alloc_sbuf_tensor("ones_c", [NWINR, 1], F32).ap()   # partition-red
    ones_cb = nc.alloc_sbuf_tensor("ones_cb", [NWINR, 1], BF16).ap()
    cst = nc.alloc_sbuf_tensor("cst", [K, 2], F32).ap()            # bias constants
    # combine scratch (persistent, written by gpsimd during the chain)
    prodz = nc.alloc_sbuf_tensor("prodz", [NWINR, W * BSH], F32).ap()
    redz = nc.alloc_sbuf_tensor("redz", [NWINR, BSH], BF16).ap()
    mwin = nc.alloc_sbuf_tensor("mwin", [NWINR, BSH], F32).ap()
    prodt = nc.alloc_sbuf_tensor("prodt", [NWINR, W * BSH], F32).ap()
    mcopy = nc.alloc_sbuf_tensor("mcopy", [NWINR, W * BSH], F32).ap()
    redt = nc.alloc_sbuf_tensor("redt", [NWINR, BSH], F32).ap()
    tmpd = nc.alloc_sbuf_tensor("tmpd", [NWINR, BSH], F32).ap()
    dconst = nc.alloc_sbuf_tensor("dconst", [NWINR, BSH], F32).ap()
    negc = nc.alloc_sbuf_tensor("negc", [1, BSH], F32).ap()
    xc = nc.alloc_sbuf_tensor("xc", [NWINR, BSH], F32).ap()

    def halving_tree_ops(buf, out, r0, r1):
        """Free-axis sum of buf[r0:r1, 0:W*BSH] -> out[r0:r1, :] as 4
        in-place halving adds (gpsimd has no free-axis reduce)."""
        ops = []
        n = W * BSH
        while n > BSH:
            h = n // 2
            dst = out[r0:r1, :] if h == BSH else buf[r0:r1, 0:h]
            ops.append((dst, buf[r0:r1, 0:h], buf[r0:r1, h:n]))
            n = h
        return ops

    # ---- pools ----
    ps_pool = ctx.enter_context(tc.tile_pool(name="ps", bufs=3, space="PSUM"))
    psb_pool = ctx.enter_context(tc.tile_pool(name="psb", bufs=1, space="PSUM"))
    psacc_pool = ctx.enter_context(tc.tile_pool(name="psa", bufs=2, space="PSUM"))
    row_pool = ctx.enter_context(tc.tile_pool(name="rows", bufs=6))
    fin_pool = ctx.enter_context(tc.tile_pool(name="fin", bufs=1))

    def w_off(t):  # column offset of chain slot t in w_all
        return ((t // W) % 2) * (W * BSH) + (t % W) * BSH

    def etr_sl(t):  # exp-emission tile of slot t in the ring
        o = ((t // W) % NRET) * (W * BSH) + (t % W) * BSH
        return etr[:, o : o + BSH]

    def em_sl(m):  # raw-emission slab of window m
        o = (m % NREM) * (W * BSH)
        return em[:, o : o + W * BSH]

    # ---- emission streaming (all off the critical path) ----
    def load_win(m, split=1):
        step = W // split
        for i in range(split):
            nc.sync.dma_start(
                em_sl(m)[:, i * step * BSH : (i + 1) * step * BSH],
                emits_d[:, m * W + i * step : m * W + (i + 1) * step, :]
                .rearrange("k t b -> k (t b)"),
            )

    def exp_win(m, split=1):
        o = (m % NRET) * (W * BSH)
        step = W * BSH // split
        for i in range(split):
            nc.scalar.activation(
                etr[0:K, o + i * step : o + (i + 1) * step],
                em_sl(m)[:, i * step : (i + 1) * step],
                AF.Exp,
                bias=cst[0:K, 0:1],
            )

    # ---- startup: keep the chain-critical queues (sync DMA, ACT, DVE)
    # free of anything the chain prologue doesn't need ----
    tr_t = fin_pool.tile([K, K], F32)
    nc.sync.dma_start(tr_t[:], trans_d)
    a0_t = fin_pool.tile([K, 1], F32)
    nc.sync.dma_start(a0_t[:], alpha0_d)
    # window 0 loads in a fine-to-coarse split so the slot-0 exp (and the
    # chain) can start as early as possible
    nc.sync.dma_start(em[:, 0:BSH],
                      emits_d[:, 0:1, :].rearrange("k t b -> k (t b)"))
    nc.sync.dma_start(em[:, BSH : 4 * BSH],
                      emits_d[:, 1:4, :].rearrange("k t b -> k (t b)"))
    for i in (1, 2, 3):
        nc.sync.dma_start(
            em[:, i * 4 * BSH : (i + 1) * 4 * BSH],
            emits_d[:, i * 4 : (i + 1) * 4, :].rearrange("k t b -> k (t b)"))
    load_win(1)

    nc.gpsimd.memset(cst[:, 0:1], 0.0)
    nc.gpsimd.memset(cst[:, 1:2], -DELTA)
    nc.gpsimd.memset(ones_r[:, :], 1.0)
    nc.gpsimd.memset(etr[K : K + 1, :], 1.0)       # ones row of the whole ring

    # dummy activation: triggers the 1.3us ACT table load before the real
    # dependencies (trans DMA) are ready
    dmy = row_pool.tile([1, 1], F32, tag="dmy")
    nc.scalar.activation(dmy[:], cst[0:1, 0:1], AF.Exp, bias=cst[0:1, 0:1])
    nc.scalar.activation(ets[:, 0:K], tr_t[:], AF.Exp, bias=cst[0:K, 1:2])
    # slot 0 absorbs alpha_0 as a per-partition bias: w_0 = exp(em_0 + a0),
    # so there is no separate t=0 state op at all - the first matmul reads
    # the slot-0 emission tile directly
    nc.scalar.activation(etr[0:K, 0:BSH], em[:, 0:BSH], AF.Exp,
                         bias=a0_t[0:K, 0:1])
    nc.scalar.activation(etr[0:K, BSH : W * BSH // 4],
                         em[:, BSH : W * BSH // 4], AF.Exp,
                         bias=cst[0:K, 0:1])
    for i in (1, 2, 3):
        nc.scalar.activation(
            etr[0:K, i * (W * BSH // 4) : (i + 1) * (W * BSH // 4)],
            em[:, i * (W * BSH // 4) : (i + 1) * (W * BSH // 4)],
            AF.Exp, bias=cst[0:K, 0:1],
        )
    exp_win(1)
    load_win(2)
    exp_win(2)
    load_win(3)

    nc.vector.memset(ets[:, K : K + 1], 1.0)
    nc.vector.memset(w_all[K : K + 1, 0:BSH], 0.0)  # slot 0 has no colsum

    # remaining setup runs on gpsimd/sync during the early chain (anything
    # the window-0 renorm booking doesn't need is deferred into the loop)
    nc.gpsimd.memset(c_rows[:, :], 0.0)
    nc.gpsimd.memset(zrow[:, :], 0.0)
    nc.gpsimd.memset(mk_u8[:, :], 0)
    # mask (one-hot over t, per b) -> slot layout: slot s <-> t = s-1.
    # maskw[win, tw*BSH + b] = mask[win*W + tw - 1, b]
    nc.sync.dma_start(
        mk_u8[0:1, BSH : W * BSH],
        mask_d[0 : W - 1].rearrange("(o t) b -> o (t b)", o=1),
    )
    nc.sync.dma_start(
        mk_u8[1 : NWINCHAIN - 1, :],
        mask_d[W - 1 : (NWINCHAIN - 1) * W - 1].rearrange(
            "(w t) b -> w t b", t=W),
    )
    nc.sync.dma_start(
        mk_u8[ROW_W31 : ROW_W31 + 1, :],
        mask_d[(NWINCHAIN - 1) * W - 1 : T - 1].rearrange(
            "(o t) b -> o (t b)", o=1),
    )
    nc.sync.dma_start(mk_u8[ROW_W32 : ROW_W32 + 1, 0:BSH], mask_d[T - 1 : T])

    accz = psacc_pool.tile([1, BSH], F32, tag="accz")
    bc_cur = None

    for t in range(1, T):
        if t == 2:
            nc.gpsimd.memset(csum[:, :], 0.0)
            nc.gpsimd.memset(c_histT[:, :], 0.0)
            nc.gpsimd.memset(ones_c[:, :], 1.0)
            nc.gpsimd.memset(ones_cb[:, :], 1.0)
            nc.gpsimd.memset(dconst[:, :], DELTA)
            nc.gpsimd.memset(negc[:, :], -2.0 * W * DELTA)
        if t == 20:
            # t value at each slot position (t = win*W + tw - 1)
            nc.gpsimd.iota(iota_i[:, :], pattern=[[1, W], [0, BSH]], base=-1,
                           channel_multiplier=W)
        if t % W == 0:
            # two windows of exp lead: the slab's WAR edge (chain readers of
            # window m-1) is already satisfied when the exp is issued, so it
            # never gates a window boundary
            m = t // W
            if m + 3 <= NWINCHAIN - 1:
                load_win(m + 3)
            if m + 2 <= NWINCHAIN - 1:
                exp_win(m + 2)
        if t % W == 5 and (t // W) % RP == RP - 1:
            # stale renorm prep (off the critical path): ln of colsum_{t-3}
            # (slot t-2's row 64), scale tile via tiny bf16 broadcast matmul;
            # the scale is folded into the group-end emission tile at t%W==8
            # and ln(s) booked into the C history rows of the NEXT group.
            win = t // W
            q = win // RP
            woff = (win % 2) * (W * BSH)
            s_row = w_all[K : K + 1, woff + 3 * BSH : woff + 4 * BSH]
            ln_s = row_pool.tile([1, BSH], F32, tag="lns")
            nc.scalar.activation(ln_s[:], s_row, AF.Ln, bias=cst[0:1, 0:1])
            rc = row_pool.tile([1, BSH], BF16, tag="recip")
            nc.scalar.activation(rc[:], ln_s[:], AF.Exp, scale=-1.0,
                                 bias=cst[0:1, 0:1])
            bc_cur = psb_pool.tile([K, BSH], F32, tag="bc")
            nc.tensor.matmul(bc_cur[:], ones_r, rc[:], start=True, stop=True)
            pw, pr = ((q + 1) % 2) * BSH, (q % 2) * BSH
            nc.gpsimd.tensor_tensor(
                c_rows[:, pw : pw + BSH], c_rows[:, pr : pr + BSH], ln_s[:],
                op=ADD,
            )
            for v in range(win + 1, min(win + 1 + RP, NWIN)):
                row = v if v <= NWINCHAIN - 2 else (
                    ROW_W31 if v == NWINCHAIN - 1 else ROW_W32)
                add2 = negc[:, :] if v == NWINCHAIN - 1 else zrow[:, :]
                nc.gpsimd.tensor_tensor(
                    c_histT[:, row * BSH : (row + 1) * BSH],
                    c_rows[:, pw : pw + BSH], add2, op=ADD,
                )
        if t % W == 8 and (t // W) % RP == RP - 1:
            # fold the renorm scale into the group-end emission tile
            # (rows 0:64 only; the ones row keeps the colsum un-scaled so
            # the boundary slot's row 64 stays in the old frame)
            sl = etr_sl((t // W) * W + W - 1)
            nc.vector.tensor_tensor(sl[0:K, :], sl[0:K, :], bc_cur[:],
                                    op=MULT)
        if t % W == 9:
            # mask-side combine precompute, one op per window on the idle
            # gpsimd engine (placed after the fold consumed bcs so it never
            # delays the broadcast of the next window)
            m = t // W
            if m == 1:
                nc.scalar.copy(maskw[:, :], mk_u8[:, :])
            elif m == 2:
                nc.scalar.copy(iotaw[:, :], iota_i[:, :])
            elif m == 3:
                nc.gpsimd.tensor_tensor(prodt[:, :], maskw[:, :],
                                        iotaw[:, :], op=MULT)
            elif 4 <= m <= 7:
                d, i0, i1 = halving_tree_ops(prodt, redt, 0, NWINR)[m - 4]
                nc.gpsimd.tensor_tensor(d, i0, i1, op=ADD)
            elif m == 8:
                nc.scalar.copy(mcopy[:, :], maskw[:, :])
            elif 9 <= m <= 12:
                d, i0, i1 = halving_tree_ops(mcopy, mwin, 0, NWINR)[m - 9]
                nc.gpsimd.tensor_tensor(d, i0, i1, op=ADD)

        # chain step: two independent 32-wide sub-chains hide latency
        op_ = w_off(t - 1)
        wprev = etr_sl(0) if t == 1 else w_all
        opp = 0 if t == 1 else op_
        o = w_off(t)
        e_t = etr_sl(t)
        ps_a = ps_pool.tile([K + 1, HB], F32, tag="ps")
        nc.tensor.matmul(ps_a[:], ets[:, :], wprev[0:K, opp : opp + HB],
                         start=True, stop=True)
        ps_b = ps_pool.tile([K + 1, HB], F32, tag="ps")
        nc.tensor.matmul(ps_b[:], ets[:, :],
                         wprev[0:K, opp + HB : opp + BSH],
                         start=True, stop=True)
        nc.vector.tensor_tensor(
            w_all[0 : K + 1, o : o + HB], ps_a[:], e_t[:, 0:HB], op=MULT
        )
        nc.vector.tensor_tensor(
            w_all[0 : K + 1, o + HB : o + BSH], ps_b[:], e_t[:, HB:BSH],
            op=MULT,
        )

        if t % W == W - 1:
            # capture the window's colsum row into the history (DMA: compute
            # engines cannot write at arbitrary start partitions)
            win = t // W
            row = win if win <= NWINCHAIN - 2 else ROW_W31
            woff = (win % 2) * (W * BSH)
            nc.gpsimd.dma_start(
                csum[row : row + 1, :], w_all[K : K + 1, woff : woff + W * BSH]
            )

        if t == T - 10:
            nc.gpsimd.dma_start(
                c_hist[:, :],
                c_histT[:, :].rearrange("o (w b) -> o w b", b=BSH),
            )
        if t == T - 9:
            # masked colsum sum for the windows already captured (0..30)
            nc.gpsimd.tensor_tensor(
                prodz[0:ROW_W32, :], csum[0:ROW_W32, :],
                maskw[0:ROW_W32, :], op=MULT,
            )
            for d, i0, i1 in halving_tree_ops(prodz, redz, 0, ROW_W32):
                nc.gpsimd.tensor_tensor(d, i0, i1, op=ADD)
        if t == T - 5:
            # C terms (c_hist complete after window 31's booking at t=501)
            nc.gpsimd.tensor_tensor(xc[:, :], mwin[:, :], c_hist[:, :],
                                    op=MULT)
            nc.gpsimd.tensor_tensor(tmpd[:, :], redt[:, :], dconst[:, :],
                                    op=MULT)
            nc.gpsimd.tensor_tensor(xc[:, :], xc[:, :], tmpd[:, :], op=ADD)

    # slot 512: one extra matmul for colsum of w_{511}
    ps = ps_pool.tile([K + 1, BSH], F32, tag="ps")
    o511 = w_off(T - 1)
    nc.tensor.matmul(ps[:], ets[:, :], w_all[0:K, o511 : o511 + BSH],
                     start=True, stop=True)
    nc.scalar.copy(csum[ROW_W32 : ROW_W32 + 1, 0:BSH], ps[K : K + 1, :])

    accc = psacc_pool.tile([1, BSH], F32, tag="accc")
    nc.tensor.matmul(accc[:], ones_c, xc[:, :], start=True, stop=True)

    # ---- tail: only the two late rows (32..33, aligned base) remain ----
    nc.vector.tensor_tensor(
        prodz[ROW_W32:NWINR, :], csum[ROW_W32:NWINR, :],
        maskw[ROW_W32:NWINR, :], op=MULT,
    )
    for dst, i0, i1 in halving_tree_ops(prodz, redz, ROW_W32, NWINR):
        nc.vector.tensor_tensor(dst, i0, i1, op=ADD)
    nc.tensor.matmul(accz[:], ones_cb, redz[:, :], start=True, stop=True)
    lnz = row_pool.tile([1, BSH], F32, tag="lnz")
    nc.scalar.activation(lnz[:], accz[:], AF.Ln, bias=cst[0:1, 0:1])
    res = row_pool.tile([1, BSH], F32, tag="res")
    nc.vector.tensor_tensor(res[:], lnz[:], accc[:], op=ADD)
    nc.sync.dma_start(out_d, res[:])


_NC_CACHE = None


def _get_nc():
    global _NC_CACHE
    if _NC_CACHE is None:
        _NC_CACHE = _build_crf_nc()
    return _NC_CACHE


def _make_in_maps(np_inputs):
    emits = np.asarray(np_inputs["emits"], dtype=np.float32)
    mask_u8 = np.asarray(np_inputs["mask"]).astype(np.uint8)
    transitions = np.asarray(np_inputs["transitions"], dtype=np.float32)
    alpha_0 = np.asarray(np_inputs["alpha_0"], dtype=np.float32)
    in_maps = []
    for c in range(NCORES):
        sl = slice(c * BSH, (c + 1) * BSH)
        in_maps.append(
            {
                # host-side transpose to [K, T, BSH]: the device reads
                # contiguous per-partition slabs already in state layout
                "emits_t": np.ascontiguousarray(
                    emits[:, sl, :].transpose(2, 0, 1)
                ),
                "maskb": np.ascontiguousarray(mask_u8[:, sl]),
                "transitions": transitions,
                "alpha_0": alpha_0,
            }
        )
    return in_maps


def kernel(emits, mask, transitions, alpha_0):
    nc = _get_nc()
    in_maps = _make_in_maps(
        {"emits": emits, "mask": mask, "transitions": transitions,
         "alpha_0": alpha_0}
    )
    res = run_bass_kernel_spmd(nc, in_maps, core_ids=list(range(NCORES)))
    total = np.float64(0.0)
    for r in res.results:
        total += np.asarray(r["out_row"], dtype=np.float64).sum()
    return np.float32(total)
